# revision 41
# baseline (speedup 1.0000x reference)
"""Depthwise 3D conv (3x3x3, SAME, C=64) on 8 Trainium2 NeuronCores.

Strategy
--------
Data-parallel over (batch, h-half): core k handles b = k//2 and output
rows h in [56*(k%2), 56*(k%2)+56), all 16 d frames. Both the d-halo and
h-halo are materialized on host (zero-padded at volume edges), so every
core runs an identical program.

TensorE mapping: partitions carry a (d, h) block — input block (8, 16)
= 128 partitions, output block (6, 14) = 84 partitions — and the
stationary operand is a per-(channel, kw) banded matrix B[(d_i, h_i),
(d_o, h_o)] = w[kd = d_i - d_o, kh = h_i - h_o, kw, c], so ONE matmul
applies 9 of the 27 taps; the 3 kw taps are w-shifts on the moving
operand's access pattern, PSUM-accumulated. d = 16 tiles as output
blocks {6, 6, 4}; the ragged 4-block uses a (6, 16) = 96-partition
input block with its own (smaller) band matrices. h = 56 tiles as 4
blocks of 14, carried in the moving free dimension alongside w
(j = 4*112 = 448).

x is host-gathered into the block-partition layout (fp16), band
matrices built on host (fp16), device output is fp16 (its ~5e-4
relative rounding is far below the fp16-input rounding already in the
products), host casts back to fp32.
"""

import json
import sys
import types

if "/opt/trn_rl_repo" not in sys.path:
    sys.path.insert(0, "/opt/trn_rl_repo")

import numpy as np

KD = KH = KW = 3
C = 64
B_FULL, D_FULL, H, W = 4, 16, 112, 112
N_CORES = 8
HH = 56  # output h rows per core
NHB = 4  # h blocks of 14 per core
HBO = 14  # out h rows per block
HBI = 16  # in h rows per block
DBO_M, DBI_M = 6, 8  # main d block: out/in frames
DBO_R, DBI_R = 4, 6  # ragged d block
D0S = [0, 6, 12]  # out-frame starts of the 3 d blocks
PM = DBI_M * HBI  # 128 in-partitions (main)
PMO = DBO_M * HBO  # 84 out-partitions (main)
PR = DBI_R * HBI  # 96 in-partitions (ragged)
PRO = DBO_R * HBO  # 56 out-partitions (ragged)
PMO_P = 128  # bm out-dim zero-padded to 128 cols so LDWEIGHTS gets FWL
CG = 4  # channels per input DMA chunk
OG = 2  # channels per output DMA chunk
F16 = np.float16

_KW_ORDER = [1, 0, 2]  # full-width tap first so PSUM start=True covers all cols


def _legalize_bir(raw: bytes) -> bytes:
    """walrus in this image caps sem waits at 1 per instruction; hoist extra
    waits onto preceding same-engine NoOps (sequencers run them in order)."""
    d = json.loads(raw)
    for fn in d["functions"]:
        for blk in fn["blocks"]:
            out = []
            for inst in blk["instructions"]:
                si = inst.get("sync_info")
                waits = (si or {}).get("on_wait") or []
                if len(waits) > 1:
                    for j, wt in enumerate(waits[:-1]):
                        out.append(
                            {
                                "debug": inst.get("debug", 0),
                                "engine": inst["engine"],
                                "ins": [],
                                "outs": [],
                                "name": f"{inst['name']}-w{j}",
                                "opcode": "NoOp",
                                "sync_info": {"on_wait": [wt], "on_update": []},
                            }
                        )
                    si["on_wait"] = [waits[-1]]
                out.append(inst)
            blk["instructions"] = out
    return json.dumps(d).encode()


def _w_ranges(kw):
    # out[w] += wt[kw] * x[w + kw - 1]
    if kw == 1:
        return 0, W, 0, W
    if kw == 0:
        return 0, W - 1, 1, W
    return 1, W, 0, W - 1


def _build_nc():
    import concourse.bass as bass
    import concourse.mybir as mybir
    import concourse.tile as tile

    nc = bass.Bass()
    xm_d = nc.declare_dram_parameter(
        "xm", [PM, C, 2, NHB, W], mybir.dt.float16, isOutput=False
    )
    xr_d = nc.declare_dram_parameter(
        "xr", [PR, C, NHB, W], mybir.dt.float16, isOutput=False
    )
    bm_d = nc.declare_dram_parameter(
        "bm", [PM, C, KW, PMO_P], mybir.dt.float16, isOutput=False
    )
    ym_d = nc.declare_dram_parameter(
        "ym", [PMO, C, 2, NHB, W], mybir.dt.float16, isOutput=True
    )
    yr_d = nc.declare_dram_parameter(
        "yr", [PRO, C, NHB, W], mybir.dt.float16, isOutput=True
    )

    with tile.TileContext(nc) as tc:
        with (
            tc.tile_pool(name="xin", bufs=5) as xin_pool,
            tc.tile_pool(name="bmat", bufs=5) as b_pool,
            tc.tile_pool(name="psum", bufs=5, space="PSUM") as psum_pool,
            tc.tile_pool(name="psumr", bufs=3, space="PSUM") as psumr_pool,
            tc.tile_pool(name="osb", bufs=3) as osb_pool,
        ):
            # warm the PE (HAM un-throttle) during the first DMA wait:
            # matmuls on uninitialized SBUF, result discarded
            warm = xin_pool.tile([PM, 448], mybir.dt.float16, tag="warm")
            wps = psum_pool.tile([PMO_P, NHB, W], mybir.dt.float32, tag="psm")
            for wi_ in range(20):
                nc.tensor.matmul(
                    wps[:, :, :],
                    warm[:, :PMO_P],
                    warm[:, :448],
                    start=True,
                    stop=True,
                    skip_group_check=True,
                )
            nc.vector.tensor_copy(warm[:, :W], wps[:, 0])

            sizes = [2, 2, 4] + [CG] * ((C - 8) // CG)
            assert sum(sizes) == C
            chunks = []
            c0 = 0
            for sz in sizes:
                chunks.append((c0, sz))
                c0 += sz
            for c0, csz in chunks:
                xm = xin_pool.tile([PM, CG, 2, NHB, W], mybir.dt.float16, tag="xm")
                xr = xin_pool.tile([PR, CG, NHB, W], mybir.dt.float16, tag="xr")
                bm = b_pool.tile([PM, CG, KW, PMO_P], mybir.dt.float16, tag="bm")
                nc.sync.dma_start(out=bm[:, :csz], in_=bm_d[:, c0 : c0 + csz])
                nc.sync.dma_start(out=xm[:, :csz], in_=xm_d[:, c0 : c0 + csz])
                nc.sync.dma_start(out=xr[:, :csz], in_=xr_d[:, c0 : c0 + csz])
                out_g = OG
                for oi in range((csz + out_g - 1) // out_g):
                    og = min(out_g, csz - oi * out_g)
                    osm = osb_pool.tile([PMO, OG, 2, NHB, W], mybir.dt.float16, tag="osm")
                    osr = osb_pool.tile([PRO, OG, NHB, W], mybir.dt.float16, tag="osr")
                    for ci in range(og):
                        cc = oi * out_g + ci
                        ps0 = psum_pool.tile([PMO_P, NHB, W], mybir.dt.float32, tag="psm")
                        ps1 = psum_pool.tile([PMO_P, NHB, W], mybir.dt.float32, tag="psm")
                        for i, kw in enumerate(_KW_ORDER):
                            wi, wj, wo, wp = _w_ranges(kw)
                            for db, ps in ((0, ps0), (1, ps1)):
                                nc.tensor.matmul(
                                    ps[:, :, wo:wp],
                                    bm[:, cc, kw, :],
                                    xm[:, cc, db, :, wi:wj],
                                    start=(i == 0),
                                    stop=(i == KW - 1),
                                    skip_group_check=(i != 0),
                                )
                        nc.vector.tensor_copy(osm[:, ci, 0], ps0[:PMO])
                        nc.scalar.copy(out=osm[:, ci, 1], in_=ps1[:PMO])
                        psr = psumr_pool.tile([PRO, NHB, W], mybir.dt.float32, tag="psr")
                        for i, kw in enumerate(_KW_ORDER):
                            wi, wj, wo, wp = _w_ranges(kw)
                            nc.tensor.matmul(
                                psr[:, :, wo:wp],
                                bm[:PR, cc, kw, :PRO],
                                xr[:, cc, :, wi:wj],
                                start=(i == 0),
                                stop=(i == KW - 1),
                                skip_group_check=(i != 0),
                            )
                        nc.vector.tensor_copy(osr[:, ci], psr[:])
                    yc0 = c0 + oi * out_g
                    nc.scalar.dma_start(out=ym_d[:, yc0 : yc0 + og], in_=osm[:, :og])
                    nc.scalar.dma_start(out=yr_d[:, yc0 : yc0 + og], in_=osr[:, :og])

    orig_to_json = nc.to_json_bytes
    nc.to_json_bytes = types.MethodType(lambda self: _legalize_bir(orig_to_json()), nc)
    return nc


def _band(wt, kw, dbi, dbo):
    """[dbi*16, C, dbo*14] band matrix for one kw: B[(d_i,h_i), c, (d_o,h_o)]
    = wt[d_i-d_o, h_i-h_o, kw, c]."""
    out = np.zeros((dbi * HBI, C, dbo * HBO), np.float32)
    do = np.arange(dbo)
    ho = np.arange(HBO)
    po = (do[:, None] * HBO + ho[None, :]).ravel()
    for kd in range(KD):
        for kh in range(KH):
            pi = ((do[:, None] + kd) * HBI + ho[None, :] + kh).ravel()
            out[pi, :, po] = wt[kd, kh, kw, :]
    return out


def _host_prep(x: np.ndarray, w: np.ndarray):
    # x: (4, 16, 112, 112, 64) f32; w: (3, 3, 3, 1, 64) f32
    wt = w[:, :, :, 0, :].astype(np.float32)  # (kd, kh, kw, c)
    bm = np.stack(
        [_band(wt, kw, DBI_M, DBO_M) for kw in range(KW)], axis=2
    )  # [PM, C, KW, PMO]; its [:96, :, :, :56] slice IS the ragged-block
    # band (same 16/14 partition/output strides, same taps)
    bm = np.concatenate(
        [bm, np.zeros((PM, C, KW, PMO_P - PMO), np.float32)], axis=3
    ).astype(F16)

    xt = np.transpose(x, (0, 4, 1, 2, 3))  # (b, c, d, h, w)

    in_maps = []
    for k in range(N_CORES):
        b = k // 2
        h0 = (k % 2) * HH
        # padded input volume: d 18 (1 zero frame each side), h 58
        xp = np.zeros((C, D_FULL + 2, HH + 2, W), np.float32)
        hlo, hhi = h0 - 1, h0 + HH + 1
        chlo, chhi = max(hlo, 0), min(hhi, H)
        xp[:, 1 : D_FULL + 1, chlo - hlo : chlo - hlo + (chhi - chlo), :] = xt[
            b, :, :, chlo:chhi, :
        ]
        xm = np.empty((PM, C, 2, NHB, W), np.float32)
        xr = np.empty((PR, C, NHB, W), np.float32)
        for db in range(2):
            for hb in range(NHB):
                blk = xp[:, D0S[db] : D0S[db] + DBI_M, hb * HBO : hb * HBO + HBI, :]
                xm[:, :, db, hb, :] = blk.transpose(1, 2, 0, 3).reshape(PM, C, W)
        for hb in range(NHB):
            blk = xp[:, D0S[2] : D0S[2] + DBI_R, hb * HBO : hb * HBO + HBI, :]
            xr[:, :, hb, :] = blk.transpose(1, 2, 0, 3).reshape(PR, C, W)
        in_maps.append(
            {"xm": xm.astype(F16), "xr": xr.astype(F16), "bm": bm}
        )
    return in_maps


def _assemble(results):
    y = np.empty((B_FULL, D_FULL, H, W, C), np.float32)
    for k in range(N_CORES):
        b = k // 2
        h0 = (k % 2) * HH
        ym = results[k]["ym"].astype(np.float32)  # [84, C, 2, 4, W]
        yr = results[k]["yr"].astype(np.float32)  # [56, C, 4, W]
        for db in range(2):
            for hb in range(NHB):
                blk = ym[:, :, db, hb, :].reshape(DBO_M, HBO, C, W)
                y[b, D0S[db] : D0S[db] + DBO_M, h0 + hb * HBO : h0 + (hb + 1) * HBO] = (
                    blk.transpose(0, 1, 3, 2)
                )
        for hb in range(NHB):
            blk = yr[:, :, hb, :].reshape(DBO_R, HBO, C, W)
            y[b, D0S[2] : D0S[2] + DBO_R, h0 + hb * HBO : h0 + (hb + 1) * HBO] = (
                blk.transpose(0, 1, 3, 2)
            )
    return y


def _run(x: np.ndarray, w: np.ndarray, trace: bool = False):
    from concourse.bass_utils import run_bass_kernel_spmd

    in_maps = _host_prep(np.asarray(x), np.asarray(w))
    last_err = None
    for attempt in range(3):
        nc = _build_nc()
        try:
            res = run_bass_kernel_spmd(nc, in_maps, list(range(N_CORES)), trace=trace)
            return _assemble(res.results), res.exec_time_ns
        except Exception as e:  # wedged device is transient; retry fresh
            last_err = e
            print(f"kernel run attempt {attempt} failed: {e!r}", file=sys.stderr)
    raise last_err


def kernel(x: np.ndarray, w: np.ndarray) -> np.ndarray:
    y, _ = _run(x, w, trace=False)
    return y


# revision 42
# speedup vs baseline: 1.0013x; 1.0013x over previous
"""Depthwise 3D conv (3x3x3, SAME, C=64) on 8 Trainium2 NeuronCores.

Strategy
--------
Data-parallel over (batch, h-half): core k handles b = k//2 and output
rows h in [56*(k%2), 56*(k%2)+56), all 16 d frames. Both the d-halo and
h-halo are materialized on host (zero-padded at volume edges), so every
core runs an identical program.

TensorE mapping: partitions carry a (d, h) block — input block (8, 16)
= 128 partitions, output block (6, 14) = 84 partitions — and the
stationary operand is a per-(channel, kw) banded matrix B[(d_i, h_i),
(d_o, h_o)] = w[kd = d_i - d_o, kh = h_i - h_o, kw, c], so ONE matmul
applies 9 of the 27 taps; the 3 kw taps are w-shifts on the moving
operand's access pattern, PSUM-accumulated. d = 16 tiles as output
blocks {6, 6, 4}; the ragged 4-block uses a (6, 16) = 96-partition
input block with its own (smaller) band matrices. h = 56 tiles as 4
blocks of 14, carried in the moving free dimension alongside w
(j = 4*112 = 448).

x is host-gathered into the block-partition layout (fp16), band
matrices built on host (fp16), device output is fp16 (its ~5e-4
relative rounding is far below the fp16-input rounding already in the
products), host casts back to fp32.
"""

import json
import sys
import types

if "/opt/trn_rl_repo" not in sys.path:
    sys.path.insert(0, "/opt/trn_rl_repo")

import numpy as np

KD = KH = KW = 3
C = 64
B_FULL, D_FULL, H, W = 4, 16, 112, 112
N_CORES = 8
HH = 56  # output h rows per core
NHB = 4  # h blocks of 14 per core
HBO = 14  # out h rows per block
HBI = 16  # in h rows per block
DBO_M, DBI_M = 6, 8  # main d block: out/in frames
DBO_R, DBI_R = 4, 6  # ragged d block
D0S = [0, 6, 12]  # out-frame starts of the 3 d blocks
PM = DBI_M * HBI  # 128 in-partitions (main)
PMO = DBO_M * HBO  # 84 out-partitions (main)
PR = DBI_R * HBI  # 96 in-partitions (ragged)
PRO = DBO_R * HBO  # 56 out-partitions (ragged)
PMO_P = 128  # bm out-dim zero-padded to 128 cols so LDWEIGHTS gets FWL
CG = 4  # channels per input DMA chunk
OG = 2  # channels per output DMA chunk
F16 = np.float16

_KW_ORDER = [1, 0, 2]  # full-width tap first so PSUM start=True covers all cols


def _legalize_bir(raw: bytes) -> bytes:
    """walrus in this image caps sem waits at 1 per instruction; hoist extra
    waits onto preceding same-engine NoOps (sequencers run them in order)."""
    d = json.loads(raw)
    for fn in d["functions"]:
        for blk in fn["blocks"]:
            out = []
            for inst in blk["instructions"]:
                si = inst.get("sync_info")
                waits = (si or {}).get("on_wait") or []
                if len(waits) > 1:
                    for j, wt in enumerate(waits[:-1]):
                        out.append(
                            {
                                "debug": inst.get("debug", 0),
                                "engine": inst["engine"],
                                "ins": [],
                                "outs": [],
                                "name": f"{inst['name']}-w{j}",
                                "opcode": "NoOp",
                                "sync_info": {"on_wait": [wt], "on_update": []},
                            }
                        )
                    si["on_wait"] = [waits[-1]]
                out.append(inst)
            blk["instructions"] = out
    return json.dumps(d).encode()


def _w_ranges(kw):
    # out[w] += wt[kw] * x[w + kw - 1]
    if kw == 1:
        return 0, W, 0, W
    if kw == 0:
        return 0, W - 1, 1, W
    return 1, W, 0, W - 1


def _build_nc():
    import concourse.bass as bass
    import concourse.mybir as mybir
    import concourse.tile as tile

    nc = bass.Bass()
    xm_d = nc.declare_dram_parameter(
        "xm", [PM, C, 2, NHB, W], mybir.dt.float16, isOutput=False
    )
    xr_d = nc.declare_dram_parameter(
        "xr", [PR, C, NHB, W], mybir.dt.float16, isOutput=False
    )
    bm_d = nc.declare_dram_parameter(
        "bm", [PM, C, KW, PMO_P], mybir.dt.float16, isOutput=False
    )
    ym_d = nc.declare_dram_parameter(
        "ym", [PMO, C, 2, NHB, W], mybir.dt.float16, isOutput=True
    )
    yr_d = nc.declare_dram_parameter(
        "yr", [PRO, C, NHB, W], mybir.dt.float16, isOutput=True
    )

    with tile.TileContext(nc) as tc:
        with (
            tc.tile_pool(name="xin", bufs=5) as xin_pool,
            tc.tile_pool(name="bmat", bufs=5) as b_pool,
            tc.tile_pool(name="psum", bufs=5, space="PSUM") as psum_pool,
            tc.tile_pool(name="psumr", bufs=3, space="PSUM") as psumr_pool,
            tc.tile_pool(name="osb", bufs=3) as osb_pool,
        ):
            # warm the PE (HAM un-throttle) during the first DMA wait:
            # matmuls on uninitialized SBUF, result discarded
            warm = xin_pool.tile([PM, 448], mybir.dt.float16, tag="warm")
            wps = psum_pool.tile([PMO_P, NHB, W], mybir.dt.float32, tag="psm")
            for wi_ in range(20):
                nc.tensor.matmul(
                    wps[:, :, :],
                    warm[:, :PMO_P],
                    warm[:, :448],
                    start=True,
                    stop=True,
                    skip_group_check=True,
                )
            nc.vector.tensor_copy(warm[:, :W], wps[:, 0])

            sizes = [2, 2, 4] + [CG] * ((C - 8) // CG)
            assert sum(sizes) == C
            chunks = []
            c0 = 0
            for sz in sizes:
                chunks.append((c0, sz))
                c0 += sz
            for c0, csz in chunks:
                xm = xin_pool.tile([PM, CG, 2, NHB, W], mybir.dt.float16, tag="xm")
                xr = xin_pool.tile([PR, CG, NHB, W], mybir.dt.float16, tag="xr")
                bm = b_pool.tile([PM, CG, KW, PMO_P], mybir.dt.float16, tag="bm")
                nc.sync.dma_start(out=bm[:, :csz], in_=bm_d[:, c0 : c0 + csz])
                nc.sync.dma_start(out=xm[:, :csz], in_=xm_d[:, c0 : c0 + csz])
                nc.sync.dma_start(out=xr[:, :csz], in_=xr_d[:, c0 : c0 + csz])
                out_g = OG
                for oi in range((csz + out_g - 1) // out_g):
                    og = min(out_g, csz - oi * out_g)
                    osm = osb_pool.tile([PMO, OG, 2, NHB, W], mybir.dt.float16, tag="osm")
                    osr = osb_pool.tile([PRO, OG, NHB, W], mybir.dt.float16, tag="osr")
                    for ci in range(og):
                        cc = oi * out_g + ci
                        ps0 = psum_pool.tile([PMO_P, NHB, W], mybir.dt.float32, tag="psm")
                        ps1 = psum_pool.tile([PMO_P, NHB, W], mybir.dt.float32, tag="psm")
                        for i, kw in enumerate(_KW_ORDER):
                            wi, wj, wo, wp = _w_ranges(kw)
                            for db, ps in ((0, ps0), (1, ps1)):
                                nc.tensor.matmul(
                                    ps[:, :, wo:wp],
                                    bm[:, cc, kw, :],
                                    xm[:, cc, db, :, wi:wj],
                                    start=(i == 0),
                                    stop=(i == KW - 1),
                                    skip_group_check=(i != 0),
                                )
                        nc.vector.tensor_copy(osm[:, ci, 0], ps0[:PMO])
                        nc.scalar.copy(out=osm[:, ci, 1], in_=ps1[:PMO])
                        psr = psumr_pool.tile([PMO_P, NHB, W], mybir.dt.float32, tag="psr")
                        for i, kw in enumerate(_KW_ORDER):
                            wi, wj, wo, wp = _w_ranges(kw)
                            nc.tensor.matmul(
                                psr[:, :, wo:wp],
                                bm[:PR, cc, kw, :],
                                xr[:, cc, :, wi:wj],
                                start=(i == 0),
                                stop=(i == KW - 1),
                                skip_group_check=(i != 0),
                            )
                        nc.vector.tensor_copy(osr[:, ci], psr[:PRO])
                    yc0 = c0 + oi * out_g
                    nc.scalar.dma_start(out=ym_d[:, yc0 : yc0 + og], in_=osm[:, :og])
                    nc.scalar.dma_start(out=yr_d[:, yc0 : yc0 + og], in_=osr[:, :og])

    orig_to_json = nc.to_json_bytes
    nc.to_json_bytes = types.MethodType(lambda self: _legalize_bir(orig_to_json()), nc)
    return nc


def _band(wt, kw, dbi, dbo):
    """[dbi*16, C, dbo*14] band matrix for one kw: B[(d_i,h_i), c, (d_o,h_o)]
    = wt[d_i-d_o, h_i-h_o, kw, c]."""
    out = np.zeros((dbi * HBI, C, dbo * HBO), np.float32)
    do = np.arange(dbo)
    ho = np.arange(HBO)
    po = (do[:, None] * HBO + ho[None, :]).ravel()
    for kd in range(KD):
        for kh in range(KH):
            pi = ((do[:, None] + kd) * HBI + ho[None, :] + kh).ravel()
            out[pi, :, po] = wt[kd, kh, kw, :]
    return out


def _host_prep(x: np.ndarray, w: np.ndarray):
    # x: (4, 16, 112, 112, 64) f32; w: (3, 3, 3, 1, 64) f32
    wt = w[:, :, :, 0, :].astype(np.float32)  # (kd, kh, kw, c)
    bm = np.stack(
        [_band(wt, kw, DBI_M, DBO_M) for kw in range(KW)], axis=2
    )  # [PM, C, KW, PMO]; its [:96, :, :, :56] slice IS the ragged-block
    # band (same 16/14 partition/output strides, same taps)
    bm = np.concatenate(
        [bm, np.zeros((PM, C, KW, PMO_P - PMO), np.float32)], axis=3
    ).astype(F16)

    xt = np.transpose(x, (0, 4, 1, 2, 3))  # (b, c, d, h, w)

    in_maps = []
    for k in range(N_CORES):
        b = k // 2
        h0 = (k % 2) * HH
        # padded input volume: d 18 (1 zero frame each side), h 58
        xp = np.zeros((C, D_FULL + 2, HH + 2, W), np.float32)
        hlo, hhi = h0 - 1, h0 + HH + 1
        chlo, chhi = max(hlo, 0), min(hhi, H)
        xp[:, 1 : D_FULL + 1, chlo - hlo : chlo - hlo + (chhi - chlo), :] = xt[
            b, :, :, chlo:chhi, :
        ]
        xm = np.empty((PM, C, 2, NHB, W), np.float32)
        xr = np.empty((PR, C, NHB, W), np.float32)
        for db in range(2):
            for hb in range(NHB):
                blk = xp[:, D0S[db] : D0S[db] + DBI_M, hb * HBO : hb * HBO + HBI, :]
                xm[:, :, db, hb, :] = blk.transpose(1, 2, 0, 3).reshape(PM, C, W)
        for hb in range(NHB):
            blk = xp[:, D0S[2] : D0S[2] + DBI_R, hb * HBO : hb * HBO + HBI, :]
            xr[:, :, hb, :] = blk.transpose(1, 2, 0, 3).reshape(PR, C, W)
        in_maps.append(
            {"xm": xm.astype(F16), "xr": xr.astype(F16), "bm": bm}
        )
    return in_maps


def _assemble(results):
    y = np.empty((B_FULL, D_FULL, H, W, C), np.float32)
    for k in range(N_CORES):
        b = k // 2
        h0 = (k % 2) * HH
        ym = results[k]["ym"].astype(np.float32)  # [84, C, 2, 4, W]
        yr = results[k]["yr"].astype(np.float32)  # [56, C, 4, W]
        for db in range(2):
            for hb in range(NHB):
                blk = ym[:, :, db, hb, :].reshape(DBO_M, HBO, C, W)
                y[b, D0S[db] : D0S[db] + DBO_M, h0 + hb * HBO : h0 + (hb + 1) * HBO] = (
                    blk.transpose(0, 1, 3, 2)
                )
        for hb in range(NHB):
            blk = yr[:, :, hb, :].reshape(DBO_R, HBO, C, W)
            y[b, D0S[2] : D0S[2] + DBO_R, h0 + hb * HBO : h0 + (hb + 1) * HBO] = (
                blk.transpose(0, 1, 3, 2)
            )
    return y


def _run(x: np.ndarray, w: np.ndarray, trace: bool = False):
    from concourse.bass_utils import run_bass_kernel_spmd

    in_maps = _host_prep(np.asarray(x), np.asarray(w))
    last_err = None
    for attempt in range(3):
        nc = _build_nc()
        try:
            res = run_bass_kernel_spmd(nc, in_maps, list(range(N_CORES)), trace=trace)
            return _assemble(res.results), res.exec_time_ns
        except Exception as e:  # wedged device is transient; retry fresh
            last_err = e
            print(f"kernel run attempt {attempt} failed: {e!r}", file=sys.stderr)
    raise last_err


def kernel(x: np.ndarray, w: np.ndarray) -> np.ndarray:
    y, _ = _run(x, w, trace=False)
    return y


# revision 43
# speedup vs baseline: 1.0082x; 1.0069x over previous
"""Depthwise 3D conv (3x3x3, SAME, C=64) on 8 Trainium2 NeuronCores.

Strategy
--------
Data-parallel over (batch, h-half): core k handles b = k//2 and output
rows h in [56*(k%2), 56*(k%2)+56), all 16 d frames. Both the d-halo and
h-halo are materialized on host (zero-padded at volume edges), so every
core runs an identical program.

TensorE mapping: partitions carry a (d, h) block — input block (8, 16)
= 128 partitions, output block (6, 14) = 84 partitions — and the
stationary operand is a per-(channel, kw) banded matrix B[(d_i, h_i),
(d_o, h_o)] = w[kd = d_i - d_o, kh = h_i - h_o, kw, c], so ONE matmul
applies 9 of the 27 taps; the 3 kw taps are w-shifts on the moving
operand's access pattern, PSUM-accumulated. d = 16 tiles as output
blocks {6, 6, 4}; the ragged 4-block uses a (6, 16) = 96-partition
input block with its own (smaller) band matrices. h = 56 tiles as 4
blocks of 14, carried in the moving free dimension alongside w
(j = 4*112 = 448).

x is host-gathered into the block-partition layout (fp16), band
matrices built on host (fp16), device output is fp16 (its ~5e-4
relative rounding is far below the fp16-input rounding already in the
products), host casts back to fp32.
"""

import json
import sys
import types

if "/opt/trn_rl_repo" not in sys.path:
    sys.path.insert(0, "/opt/trn_rl_repo")

import numpy as np

KD = KH = KW = 3
C = 64
B_FULL, D_FULL, H, W = 4, 16, 112, 112
N_CORES = 8
HH = 56  # output h rows per core
NHB = 4  # h blocks of 14 per core
HBO = 14  # out h rows per block
HBI = 16  # in h rows per block
DBO_M, DBI_M = 6, 8  # main d block: out/in frames
DBO_R, DBI_R = 4, 6  # ragged d block
D0S = [0, 6, 12]  # out-frame starts of the 3 d blocks
PM = DBI_M * HBI  # 128 in-partitions (main)
PMO = DBO_M * HBO  # 84 out-partitions (main)
PR = DBI_R * HBI  # 96 in-partitions (ragged)
PRO = DBO_R * HBO  # 56 out-partitions (ragged)
PMO_P = 128  # bm out-dim zero-padded to 128 cols so LDWEIGHTS gets FWL
CG = 4  # channels per input DMA chunk
OG = 2  # channels per output DMA chunk
F16 = np.float16

_KW_ORDER = [1, 0, 2]  # full-width tap first so PSUM start=True covers all cols


def _legalize_bir(raw: bytes) -> bytes:
    """walrus in this image caps sem waits at 1 per instruction; hoist extra
    waits onto preceding same-engine NoOps (sequencers run them in order)."""
    d = json.loads(raw)
    for fn in d["functions"]:
        for blk in fn["blocks"]:
            out = []
            for inst in blk["instructions"]:
                si = inst.get("sync_info")
                waits = (si or {}).get("on_wait") or []
                if len(waits) > 1:
                    for j, wt in enumerate(waits[:-1]):
                        out.append(
                            {
                                "debug": inst.get("debug", 0),
                                "engine": inst["engine"],
                                "ins": [],
                                "outs": [],
                                "name": f"{inst['name']}-w{j}",
                                "opcode": "NoOp",
                                "sync_info": {"on_wait": [wt], "on_update": []},
                            }
                        )
                    si["on_wait"] = [waits[-1]]
                out.append(inst)
            blk["instructions"] = out
    return json.dumps(d).encode()


def _w_ranges(kw):
    # out[w] += wt[kw] * x[w + kw - 1]
    if kw == 1:
        return 0, W, 0, W
    if kw == 0:
        return 0, W - 1, 1, W
    return 1, W, 0, W - 1


def _build_nc():
    import concourse.bass as bass
    import concourse.mybir as mybir
    import concourse.tile as tile

    nc = bass.Bass()
    xm_d = nc.declare_dram_parameter(
        "xm", [PM, C, 2, NHB, W], mybir.dt.float16, isOutput=False
    )
    xr_d = nc.declare_dram_parameter(
        "xr", [PR, C, NHB, W], mybir.dt.float16, isOutput=False
    )
    bm_d = nc.declare_dram_parameter(
        "bm", [PM, C, KW, PMO], mybir.dt.float16, isOutput=False
    )
    ym_d = nc.declare_dram_parameter(
        "ym", [PMO, C, 2, NHB, W], mybir.dt.float16, isOutput=True
    )
    yr_d = nc.declare_dram_parameter(
        "yr", [PRO, C, NHB, W], mybir.dt.float16, isOutput=True
    )

    with tile.TileContext(nc) as tc:
        with (
            tc.tile_pool(name="xin", bufs=5) as xin_pool,
            tc.tile_pool(name="bmat", bufs=5) as b_pool,
            tc.tile_pool(name="psum", bufs=5, space="PSUM") as psum_pool,
            tc.tile_pool(name="psumr", bufs=3, space="PSUM") as psumr_pool,
            tc.tile_pool(name="osb", bufs=3) as osb_pool,
        ):
            # warm the PE (HAM un-throttle) during the first DMA wait:
            # matmuls on uninitialized SBUF, result discarded
            warm = xin_pool.tile([PM, 448], mybir.dt.float16, tag="warm")
            wps = psum_pool.tile([PMO_P, NHB, W], mybir.dt.float32, tag="psm")
            for wi_ in range(20):
                nc.tensor.matmul(
                    wps[:, :, :],
                    warm[:, :PMO_P],
                    warm[:, :448],
                    start=True,
                    stop=True,
                    skip_group_check=True,
                )
            nc.vector.tensor_copy(warm[:, :W], wps[:, 0])

            sizes = [2, 2, 4] + [CG] * ((C - 8) // CG)
            assert sum(sizes) == C
            chunks = []
            c0 = 0
            for sz in sizes:
                chunks.append((c0, sz))
                c0 += sz
            for c0, csz in chunks:
                xm = xin_pool.tile([PM, CG, 2, NHB, W], mybir.dt.float16, tag="xm")
                xr = xin_pool.tile([PR, CG, NHB, W], mybir.dt.float16, tag="xr")
                bm = b_pool.tile([PM, CG, KW, PMO_P], mybir.dt.float16, tag="bm")
                nc.sync.dma_start(out=bm[:, :csz, :, :PMO], in_=bm_d[:, c0 : c0 + csz])
                nc.sync.dma_start(out=xm[:, :csz], in_=xm_d[:, c0 : c0 + csz])
                nc.sync.dma_start(out=xr[:, :csz], in_=xr_d[:, c0 : c0 + csz])
                out_g = OG
                for oi in range((csz + out_g - 1) // out_g):
                    og = min(out_g, csz - oi * out_g)
                    osm = osb_pool.tile([PMO, OG, 2, NHB, W], mybir.dt.float16, tag="osm")
                    osr = osb_pool.tile([PRO, OG, NHB, W], mybir.dt.float16, tag="osr")
                    for ci in range(og):
                        cc = oi * out_g + ci
                        ps0 = psum_pool.tile([PMO_P, NHB, W], mybir.dt.float32, tag="psm")
                        ps1 = psum_pool.tile([PMO_P, NHB, W], mybir.dt.float32, tag="psm")
                        for i, kw in enumerate(_KW_ORDER):
                            wi, wj, wo, wp = _w_ranges(kw)
                            for db, ps in ((0, ps0), (1, ps1)):
                                nc.tensor.matmul(
                                    ps[:, :, wo:wp],
                                    bm[:, cc, kw, :],
                                    xm[:, cc, db, :, wi:wj],
                                    start=(i == 0),
                                    stop=(i == KW - 1),
                                    skip_group_check=(i != 0),
                                )
                        nc.vector.tensor_copy(osm[:, ci, 0], ps0[:PMO])
                        nc.scalar.copy(out=osm[:, ci, 1], in_=ps1[:PMO])
                        psr = psumr_pool.tile([PMO_P, NHB, W], mybir.dt.float32, tag="psr")
                        for i, kw in enumerate(_KW_ORDER):
                            wi, wj, wo, wp = _w_ranges(kw)
                            nc.tensor.matmul(
                                psr[:, :, wo:wp],
                                bm[:PR, cc, kw, :],
                                xr[:, cc, :, wi:wj],
                                start=(i == 0),
                                stop=(i == KW - 1),
                                skip_group_check=(i != 0),
                            )
                        nc.vector.tensor_copy(osr[:, ci], psr[:PRO])
                    yc0 = c0 + oi * out_g
                    nc.scalar.dma_start(out=ym_d[:, yc0 : yc0 + og], in_=osm[:, :og])
                    nc.scalar.dma_start(out=yr_d[:, yc0 : yc0 + og], in_=osr[:, :og])

    orig_to_json = nc.to_json_bytes
    nc.to_json_bytes = types.MethodType(lambda self: _legalize_bir(orig_to_json()), nc)
    return nc


def _band(wt, kw, dbi, dbo):
    """[dbi*16, C, dbo*14] band matrix for one kw: B[(d_i,h_i), c, (d_o,h_o)]
    = wt[d_i-d_o, h_i-h_o, kw, c]."""
    out = np.zeros((dbi * HBI, C, dbo * HBO), np.float32)
    do = np.arange(dbo)
    ho = np.arange(HBO)
    po = (do[:, None] * HBO + ho[None, :]).ravel()
    for kd in range(KD):
        for kh in range(KH):
            pi = ((do[:, None] + kd) * HBI + ho[None, :] + kh).ravel()
            out[pi, :, po] = wt[kd, kh, kw, :]
    return out


def _host_prep(x: np.ndarray, w: np.ndarray):
    # x: (4, 16, 112, 112, 64) f32; w: (3, 3, 3, 1, 64) f32
    wt = w[:, :, :, 0, :].astype(np.float32)  # (kd, kh, kw, c)
    bm = np.stack(
        [_band(wt, kw, DBI_M, DBO_M) for kw in range(KW)], axis=2
    )  # [PM, C, KW, PMO]; its [:96, :, :, :56] slice IS the ragged-block
    # band (same 16/14 partition/output strides, same taps)
    bm = bm.astype(F16)

    xt = np.transpose(x, (0, 4, 1, 2, 3))  # (b, c, d, h, w)

    in_maps = []
    for k in range(N_CORES):
        b = k // 2
        h0 = (k % 2) * HH
        # padded input volume: d 18 (1 zero frame each side), h 58
        xp = np.zeros((C, D_FULL + 2, HH + 2, W), np.float32)
        hlo, hhi = h0 - 1, h0 + HH + 1
        chlo, chhi = max(hlo, 0), min(hhi, H)
        xp[:, 1 : D_FULL + 1, chlo - hlo : chlo - hlo + (chhi - chlo), :] = xt[
            b, :, :, chlo:chhi, :
        ]
        xm = np.empty((PM, C, 2, NHB, W), np.float32)
        xr = np.empty((PR, C, NHB, W), np.float32)
        for db in range(2):
            for hb in range(NHB):
                blk = xp[:, D0S[db] : D0S[db] + DBI_M, hb * HBO : hb * HBO + HBI, :]
                xm[:, :, db, hb, :] = blk.transpose(1, 2, 0, 3).reshape(PM, C, W)
        for hb in range(NHB):
            blk = xp[:, D0S[2] : D0S[2] + DBI_R, hb * HBO : hb * HBO + HBI, :]
            xr[:, :, hb, :] = blk.transpose(1, 2, 0, 3).reshape(PR, C, W)
        in_maps.append(
            {"xm": xm.astype(F16), "xr": xr.astype(F16), "bm": bm}
        )
    return in_maps


def _assemble(results):
    y = np.empty((B_FULL, D_FULL, H, W, C), np.float32)
    for k in range(N_CORES):
        b = k // 2
        h0 = (k % 2) * HH
        ym = results[k]["ym"].astype(np.float32)  # [84, C, 2, 4, W]
        yr = results[k]["yr"].astype(np.float32)  # [56, C, 4, W]
        for db in range(2):
            for hb in range(NHB):
                blk = ym[:, :, db, hb, :].reshape(DBO_M, HBO, C, W)
                y[b, D0S[db] : D0S[db] + DBO_M, h0 + hb * HBO : h0 + (hb + 1) * HBO] = (
                    blk.transpose(0, 1, 3, 2)
                )
        for hb in range(NHB):
            blk = yr[:, :, hb, :].reshape(DBO_R, HBO, C, W)
            y[b, D0S[2] : D0S[2] + DBO_R, h0 + hb * HBO : h0 + (hb + 1) * HBO] = (
                blk.transpose(0, 1, 3, 2)
            )
    return y


def _run(x: np.ndarray, w: np.ndarray, trace: bool = False):
    from concourse.bass_utils import run_bass_kernel_spmd

    in_maps = _host_prep(np.asarray(x), np.asarray(w))
    last_err = None
    for attempt in range(3):
        nc = _build_nc()
        try:
            res = run_bass_kernel_spmd(nc, in_maps, list(range(N_CORES)), trace=trace)
            return _assemble(res.results), res.exec_time_ns
        except Exception as e:  # wedged device is transient; retry fresh
            last_err = e
            print(f"kernel run attempt {attempt} failed: {e!r}", file=sys.stderr)
    raise last_err


def kernel(x: np.ndarray, w: np.ndarray) -> np.ndarray:
    y, _ = _run(x, w, trace=False)
    return y
